# revision 3
# baseline (speedup 1.0000x reference)
"""LIF spike kernel for Trainium2 (Bass/Tile), data-parallel over batch on 8 cores.

Host layout per core: x_core [C=128, T=8, F=B_loc*HW=4096] f32, so each
timestep t is one [128, 4096] tile covering all 4 local batch elements.

Per timestep t the work is split by column range into two independent chains:
  DVE cols [0:SPLIT]   : m = (u<=1)*u (STT), u' = m*TAU + x' (STT)
  Pool cols [SPLIT:F]  : s = (u<=1) (TS), m = s*u (TT), p = m*TAU (TS),
                         u' = p + x' (TT)   (Pool has no STT on core v3)
  Act engine (full width): spike = sign(u - 1) -> u8; the f32->u8 convert
  saturates (-1 -> 0, +1 -> 1) so out == (u > 1) exactly; host decodes ==1.

Input stream (16.8 MB f32/core) + output (4.2 MB u8) is the DMA roofline
(~59 us at ~358 GB/s/core); both engine chains stay under it.
"""

import numpy as np

import concourse.bacc as bacc
import concourse.mybir as mybir
from concourse.tile import TileContext
from concourse.bass_utils import run_bass_kernel_spmd

B, T, C, H, W = 32, 8, 128, 32, 32
HW = H * W
N_CORES = 8
B_LOC = B // N_CORES
F = B_LOC * HW  # 4096
TAU = 0.5
THRESH = 1.0
SPLIT = 3136  # DVE columns; Pool gets F - SPLIT

_nc_cache = None


def build_nc():
    nc = bacc.Bacc("TRN2", target_bir_lowering=False)
    f32 = mybir.dt.float32
    u8 = mybir.dt.uint8
    op = mybir.AluOpType
    AF = mybir.ActivationFunctionType
    x = nc.dram_tensor("x", [C, T, F], f32, kind="ExternalInput")
    out = nc.dram_tensor("out", [C, T, F], u8, kind="ExternalOutput")
    FP = F - SPLIT

    with TileContext(nc) as tc:
        with (
            tc.tile_pool(name="xp", bufs=5) as xp,
            tc.tile_pool(name="up", bufs=3) as up,
            tc.tile_pool(name="mp", bufs=2) as mp,
            tc.tile_pool(name="pp", bufs=2) as pp,
            tc.tile_pool(name="op_", bufs=3) as opool,
            tc.tile_pool(name="cp", bufs=1) as cp,
        ):
            neg1 = cp.tile([C, 1], f32, tag="neg1")
            nc.gpsimd.memset(neg1[:], -1.0)
            xt = []
            for t in range(T):
                xtile = xp.tile([C, F], f32, tag="x")
                nc.sync.dma_start(out=xtile[:], in_=x[:, t])
                xt.append(xtile)
            u = xt[0]
            for t in range(T):
                ot = opool.tile([C, F], u8, tag="o")
                nc.scalar.activation(ot[:], u[:], AF.Sign, bias=neg1[:], scale=1.0)
                nc.scalar.dma_start(out=out[:, t], in_=ot[:])
                if t < T - 1:
                    ud, up_ = u[:, :SPLIT], u[:, SPLIT:]
                    m = mp.tile([C, SPLIT], f32, tag="m")
                    nc.vector.scalar_tensor_tensor(
                        m[:], ud, THRESH, ud, op.is_le, op.mult
                    )
                    sp = pp.tile([C, FP], f32, tag="s")
                    nc.gpsimd.tensor_scalar(sp[:], up_, THRESH, None, op.is_le)
                    mpt = pp.tile([C, FP], f32, tag="mp")
                    nc.gpsimd.tensor_tensor(mpt[:], sp[:], up_, op.mult)
                    un = up.tile([C, F], f32, tag="u")
                    nc.vector.scalar_tensor_tensor(
                        un[:, :SPLIT], m[:], TAU, xt[t + 1][:, :SPLIT],
                        op.mult, op.add,
                    )
                    tp = pp.tile([C, FP], f32, tag="tp")
                    nc.gpsimd.tensor_scalar(tp[:], mpt[:], TAU, None, op.mult)
                    nc.gpsimd.tensor_tensor(
                        un[:, SPLIT:], tp[:], xt[t + 1][:, SPLIT:], op.add
                    )
                    u = un
    nc.compile()
    return nc


def make_in_maps(x: np.ndarray) -> list[dict]:
    # x [B, T, C, H, W] -> per core [C, T, B_loc*HW]
    xs = np.ascontiguousarray(x).reshape(B, T, C, HW)
    return [
        {
            "x": np.ascontiguousarray(
                xs[i * B_LOC : (i + 1) * B_LOC].transpose(2, 1, 0, 3)
            ).reshape(C, T, F)
        }
        for i in range(N_CORES)
    ]


def kernel(x: np.ndarray) -> np.ndarray:
    global _nc_cache
    if _nc_cache is None:
        _nc_cache = build_nc()
    res = run_bass_kernel_spmd(_nc_cache, make_in_maps(x), list(range(N_CORES)))
    # out[c, t, b_loc*HW+hw]: spike iff value == 1 (sign in u8: -1 saturates to 0)
    parts = [
        (res.results[i]["out"].reshape(C, T, B_LOC, HW) == 1).transpose(2, 1, 0, 3)
        for i in range(N_CORES)
    ]
    full = np.concatenate(parts, axis=0)
    return full.reshape(B, T, C, H, W).astype(np.float32)


# revision 4
# speedup vs baseline: 2.6102x; 2.6102x over previous
"""LIF spike kernel for Trainium2 (Bass/Tile), data-parallel over batch on 8 cores.

Host layout per core: x_core [C=128, T=8, F=B_loc*HW=4096] f32, so each
timestep t is one [128, 4096] tile covering all 4 local batch elements.

Per timestep t the work is split by column range:
  DVE cols [0:SPLIT]   : m = (u<=1)*u (STT), u' = m*TAU + x' (STT)
  Pool cols [SPLIT:F]  : DVE helper s = (u<=1)*TAU (TS, 2x mode), then
                         Pool m = s*u (TT), u' = m + x' (TT)
                         (Pool has no STT; its TENSOR_SCALAR is ~14 ns/col
                         software loop - avoid)
  Act engine (full width): spike = sign(u - 1) -> u8; the f32->u8 convert
  saturates (-1 -> 0, +1 -> 1) so out == (u > 1) exactly; host decodes ==1.

Input stream (16.8 MB f32/core) + output (4.2 MB u8) is the DMA roofline
(~59 us at ~358 GB/s/core).
"""

import numpy as np

import concourse.bacc as bacc
import concourse.mybir as mybir
from concourse.tile import TileContext
from concourse.bass_utils import run_bass_kernel_spmd

B, T, C, H, W = 32, 8, 128, 32, 32
HW = H * W
N_CORES = 8
B_LOC = B // N_CORES
F = B_LOC * HW  # 4096
TAU = 0.5
THRESH = 1.0
SPLIT = 2688  # DVE columns; Pool gets F - SPLIT

_nc_cache = None


def build_nc():
    nc = bacc.Bacc("TRN2", target_bir_lowering=False)
    f32 = mybir.dt.float32
    u8 = mybir.dt.uint8
    op = mybir.AluOpType
    AF = mybir.ActivationFunctionType
    x = nc.dram_tensor("x", [C, T, F], f32, kind="ExternalInput")
    out = nc.dram_tensor("out", [C, T, F], u8, kind="ExternalOutput")
    FP = F - SPLIT

    with TileContext(nc) as tc:
        with (
            tc.tile_pool(name="xp", bufs=5) as xp,
            tc.tile_pool(name="up", bufs=3) as up,
            tc.tile_pool(name="mp", bufs=2) as mp,
            tc.tile_pool(name="pp", bufs=2) as pp,
            tc.tile_pool(name="op_", bufs=3) as opool,
            tc.tile_pool(name="cp", bufs=1) as cp,
        ):
            neg1 = cp.tile([C, 1], f32, tag="neg1")
            nc.gpsimd.memset(neg1[:], -1.0)
            xt = []
            for t in range(T):
                xtile = xp.tile([C, F], f32, tag="x")
                nc.sync.dma_start(out=xtile[:], in_=x[:, t])
                xt.append(xtile)
            u = xt[0]
            for t in range(T):
                ot = opool.tile([C, F], u8, tag="o")
                nc.scalar.activation(ot[:], u[:], AF.Sign, bias=neg1[:], scale=1.0)
                nc.scalar.dma_start(out=out[:, t], in_=ot[:])
                if t < T - 1:
                    ud, upx = u[:, :SPLIT], u[:, SPLIT:]
                    # DVE helper for Pool cols first (unblocks Pool asap)
                    st = pp.tile([C, FP], f32, tag="s")
                    nc.vector.tensor_scalar(st[:], upx, THRESH, TAU, op.is_le, op.mult)
                    m = mp.tile([C, SPLIT], f32, tag="m")
                    nc.vector.scalar_tensor_tensor(
                        m[:], ud, THRESH, ud, op.is_le, op.mult
                    )
                    mpt = pp.tile([C, FP], f32, tag="mp")
                    nc.gpsimd.tensor_tensor(mpt[:], st[:], upx, op.mult)
                    un = up.tile([C, F], f32, tag="u")
                    nc.vector.scalar_tensor_tensor(
                        un[:, :SPLIT], m[:], TAU, xt[t + 1][:, :SPLIT],
                        op.mult, op.add,
                    )
                    nc.gpsimd.tensor_tensor(
                        un[:, SPLIT:], mpt[:], xt[t + 1][:, SPLIT:], op.add
                    )
                    u = un
    nc.compile()
    return nc


def make_in_maps(x: np.ndarray) -> list[dict]:
    # x [B, T, C, H, W] -> per core [C, T, B_loc*HW]
    xs = np.ascontiguousarray(x).reshape(B, T, C, HW)
    return [
        {
            "x": np.ascontiguousarray(
                xs[i * B_LOC : (i + 1) * B_LOC].transpose(2, 1, 0, 3)
            ).reshape(C, T, F)
        }
        for i in range(N_CORES)
    ]


def kernel(x: np.ndarray) -> np.ndarray:
    global _nc_cache
    if _nc_cache is None:
        _nc_cache = build_nc()
    res = run_bass_kernel_spmd(_nc_cache, make_in_maps(x), list(range(N_CORES)))
    # out[c, t, b_loc*HW+hw]: spike iff value == 1 (sign in u8: -1 saturates to 0)
    parts = [
        (res.results[i]["out"].reshape(C, T, B_LOC, HW) == 1).transpose(2, 1, 0, 3)
        for i in range(N_CORES)
    ]
    full = np.concatenate(parts, axis=0)
    return full.reshape(B, T, C, H, W).astype(np.float32)


# revision 5
# speedup vs baseline: 2.8479x; 1.0911x over previous
"""LIF spike kernel for Trainium2 (Bass/Tile), data-parallel over batch on 8 cores.

Host layout per core: x_core [C=128, T=8, F=B_loc*HW=4096] f32. Columns are
split into an L range (DVE) and R range (Pool); every op writes a FULL tile
(sliced writes run DVE at half rate).

Per timestep t:
  L cols (DVE):  mL = (uL<=1)*uL (STT), uL' = mL*TAU + xL' (STT)
  R cols:        sR = (uR<=1)*TAU (DVE TS, 2x mode), mR = sR*uR (Pool TT),
                 uR' = mR + xR' (Pool TT)
  Act (both):    spike = sign(u - 1) -> u8 (saturates: -1 -> 0, +1 -> 1,
                 so out == (u > 1) exactly; host decodes ==1)

Measured rates: DVE STT 1.1 ns/col (full-tile out), DVE TS 2x 0.63, Pool TT
~4.0, Act 0.905. DMA sustains ~430 GB/s, so compute chain is the bottleneck.
"""

import numpy as np

import concourse.bacc as bacc
import concourse.mybir as mybir
from concourse.tile import TileContext
from concourse.bass_utils import run_bass_kernel_spmd

B, T, C, H, W = 32, 8, 128, 32, 32
HW = H * W
N_CORES = 8
B_LOC = B // N_CORES
F = B_LOC * HW  # 4096
TAU = 0.5
THRESH = 1.0
FL = 3200  # DVE columns
FR = F - FL  # Pool columns

_nc_cache = None


def build_nc():
    nc = bacc.Bacc("TRN2", target_bir_lowering=False)
    f32 = mybir.dt.float32
    u8 = mybir.dt.uint8
    op = mybir.AluOpType
    AF = mybir.ActivationFunctionType
    x = nc.dram_tensor("x", [C, T, F], f32, kind="ExternalInput")
    out = nc.dram_tensor("out", [C, T, F], u8, kind="ExternalOutput")

    with TileContext(nc) as tc:
        with (
            tc.tile_pool(name="xlp", bufs=5) as xlp,
            tc.tile_pool(name="xrp", bufs=5) as xrp,
            tc.tile_pool(name="ulp", bufs=3) as ulp,
            tc.tile_pool(name="urp", bufs=3) as urp,
            tc.tile_pool(name="mlp", bufs=2) as mlp,
            tc.tile_pool(name="srp", bufs=2) as srp,
            tc.tile_pool(name="mrp", bufs=2) as mrp,
            tc.tile_pool(name="olp", bufs=3) as olp,
            tc.tile_pool(name="orp", bufs=3) as orp,
            tc.tile_pool(name="cp", bufs=1) as cp,
        ):
            neg1 = cp.tile([C, 1], f32, tag="neg1")
            nc.gpsimd.memset(neg1[:], -1.0)
            xl, xr = [], []
            for t in range(T):
                xt = xlp.tile([C, FL], f32, tag="xl")
                nc.sync.dma_start(out=xt[:], in_=x[:, t, :FL])
                xl.append(xt)
                xt = xrp.tile([C, FR], f32, tag="xr")
                nc.sync.dma_start(out=xt[:], in_=x[:, t, FL:])
                xr.append(xt)
            ul, ur = xl[0], xr[0]
            for t in range(T):
                ol = olp.tile([C, FL], u8, tag="ol")
                nc.scalar.activation(ol[:], ul[:], AF.Sign, bias=neg1[:], scale=1.0)
                nc.scalar.dma_start(out=out[:, t, :FL], in_=ol[:])
                orr = orp.tile([C, FR], u8, tag="or")
                nc.scalar.activation(orr[:], ur[:], AF.Sign, bias=neg1[:], scale=1.0)
                nc.scalar.dma_start(out=out[:, t, FL:], in_=orr[:])
                if t < T - 1:
                    # R chain helper first to unblock Pool
                    sr = srp.tile([C, FR], f32, tag="sr")
                    nc.vector.tensor_scalar(sr[:], ur[:], THRESH, TAU, op.is_le, op.mult)
                    ml = mlp.tile([C, FL], f32, tag="ml")
                    nc.vector.scalar_tensor_tensor(
                        ml[:], ul[:], THRESH, ul[:], op.is_le, op.mult
                    )
                    mr = mrp.tile([C, FR], f32, tag="mr")
                    nc.gpsimd.tensor_tensor(mr[:], sr[:], ur[:], op.mult)
                    unl = ulp.tile([C, FL], f32, tag="ul")
                    nc.vector.scalar_tensor_tensor(
                        unl[:], ml[:], TAU, xl[t + 1][:], op.mult, op.add
                    )
                    unr = urp.tile([C, FR], f32, tag="ur")
                    nc.gpsimd.tensor_tensor(unr[:], mr[:], xr[t + 1][:], op.add)
                    ul, ur = unl, unr
    nc.compile()
    return nc


def make_in_maps(x: np.ndarray) -> list[dict]:
    # x [B, T, C, H, W] -> per core [C, T, B_loc*HW]
    xs = np.ascontiguousarray(x).reshape(B, T, C, HW)
    return [
        {
            "x": np.ascontiguousarray(
                xs[i * B_LOC : (i + 1) * B_LOC].transpose(2, 1, 0, 3)
            ).reshape(C, T, F)
        }
        for i in range(N_CORES)
    ]


def kernel(x: np.ndarray) -> np.ndarray:
    global _nc_cache
    if _nc_cache is None:
        _nc_cache = build_nc()
    res = run_bass_kernel_spmd(_nc_cache, make_in_maps(x), list(range(N_CORES)))
    # out[c, t, b_loc*HW+hw]: spike iff value == 1 (sign in u8: -1 saturates to 0)
    parts = [
        (res.results[i]["out"].reshape(C, T, B_LOC, HW) == 1).transpose(2, 1, 0, 3)
        for i in range(N_CORES)
    ]
    full = np.concatenate(parts, axis=0)
    return full.reshape(B, T, C, H, W).astype(np.float32)
